# revision 29
# baseline (speedup 1.0000x reference)
"""Causal single-head attention (B=4, T=2048, D=1024, fp32) on 8 trn2 cores.

Sharding: each core takes one (batch, parity) pair: batch b = core//2,
parity p = core%2.  Within its batch, a core owns the query rows
{256*i + 2*j + p : i in 0..7, j in 0..127} -- i.e. 8 query tiles of 128
rows, where tile i holds every-other row of the global row range
[256*i, 256*(i+1)).  With a causal mask, tile i only needs keys
[0, 256*(i+1)), so the per-tile key length (2*(i+1) blocks of 128) is
identical for both parities -> one SPMD program, perfectly load-balanced,
and ~1.8x less matmul work than dense.

Per q-tile pipeline (per core):
  S = Q_tile @ K^T (PE).  Q and K are split host-side into fp16 hi/lo
     pairs and S is computed as qh@kh + qh@kl + ql@kh (3 fp16 passes at
     1 PE cycle/row ~= fp32 precision, vs native fp32's 4 cycles/row;
     the PE multiplies fp16 subnormals exactly and the dropped ql@kl
     term is below fp32 accumulation noise), accumulated in fp32 PSUM
     over 8 c-chunks.
  PSUM -> SBUF copy (ACT) with mask-bias add on the diagonal band (DVE,
     from the real mask input), group-wise row maxes pipelined behind
     the matmuls (DVE).
  P = exp(32*S - 32*max) (ACT, fp16 out, row-sums via accum_out)
  P^T per 128-block (PE transpose via identity) -> O += P^T.T @ V
     (PE, fp16 operands, fp32 PSUM accumulation)
  O *= 1/rowsum (DVE), DMA out.
Stage B of tile i runs on ACT/DVE while stage A (QK) of the next tile
runs on the PE, so the PE never waits on the softmax.  Warm-up matmuls
on a zeroed tile cover the initial DMA prologue and keep the PE's HAM
clock gate at full rate.

If the mask input is NOT exactly the causal triu mask, falls back to a
dense variant of the same program (all 16 key blocks per q-tile, full
mask bias applied) which is correct for any additive {0,1} mask.
"""

import os

import numpy as np

import concourse.mybir as mybir
import concourse.tile as tile
from concourse import bacc
from concourse.bass_utils import run_bass_kernel_spmd
from concourse.masks import make_identity

B, T, D = 4, 2048, 1024
NEG = -1000000000.0
P = 128          # partitions
NCORES = 8
NQT = 8          # q-tiles of 128 rows per core
CCHUNKS = D // P  # 8 contraction chunks
STILES = T // P   # 16 key tiles per batch
F32 = mybir.dt.float32
F16 = mybir.dt.float16

# AV (P @ V) operand dtype: fp16 runs the PE at 1 cycle/row vs fp32's 4.
# P in [0,1] and V ~ N(0,1) both fit fp16 with ~2^-11 relative rounding.
AV_DT = F32 if os.environ.get("KERNEL_AV_F32", "0") == "1" else F16
# QK path: fp16 hi/lo split, S = qh@kh + qh@kl + ql@kh (3 passes at
# 1 cycle/row) instead of native fp32 (4 cycles/row).  The split keeps
# ~22 mantissa bits; the PE multiplies fp16 subnormals exactly (verified
# on HW), and the dropped ql@kl term is below fp32 accumulation noise.
QK_SPLIT = os.environ.get("KERNEL_QK_F32", "0") != "1"
_cache = {}


def _tile_cfg(causal: bool):
    """Per-q-tile (s_cols, bias_off, bias_cols)."""
    if causal:
        return [(256 * (i + 1), 256 * i, 256) for i in range(NQT)]
    return [(T, 0, T) for _ in range(NQT)]


def _build(causal: bool):
    cfg = _tile_cfg(causal)
    bias_cols = cfg[0][2]

    nc = bacc.Bacc("TRN2", target_bir_lowering=False, debug=False,
                   num_devices=NCORES)
    if QK_SPLIT:
        qThl = nc.declare_dram_parameter("qThl", [D, 2, NQT * P], F16,
                                         isOutput=False)
        kThl = nc.declare_dram_parameter("kThl", [D, 2, T], F16,
                                         isOutput=False)
    else:
        qT = nc.declare_dram_parameter("qT", [D, NQT * P], F32, isOutput=False)
        kT = nc.declare_dram_parameter("kT", [D, T], F32, isOutput=False)
    v = nc.declare_dram_parameter("v", [T, D], AV_DT, isOutput=False)
    # For the causal path the diagonal-band bias block is identical for
    # every q-tile (band entry (j, u) is masked iff u > 2j + parity), so a
    # single [P, 256] input suffices; the dense path keeps per-tile rows.
    if causal:
        biasd = nc.declare_dram_parameter("bias", [P, bias_cols], F32,
                                          isOutput=False)
    else:
        biasd = nc.declare_dram_parameter("bias", [NQT, P, bias_cols], F32,
                                          isOutput=False)
    out = nc.declare_dram_parameter("out", [NQT * P, D], F32, isOutput=True)

    AX = mybir.AxisListType.X
    EXP = mybir.ActivationFunctionType.Exp

    with tile.TileContext(nc) as tc:
        with (
            tc.tile_pool(name="const", bufs=1) as constp,
            tc.tile_pool(name="kv", bufs=1) as kvp,
            tc.tile_pool(name="qt", bufs=2) as qtp,
            tc.tile_pool(name="biasp", bufs=2) as biasp,
            tc.tile_pool(name="pp", bufs=2) as pp,
            tc.tile_pool(name="ssb", bufs=2) as ssbp,
            tc.tile_pool(name="ptp", bufs=3) as ptp,
            tc.tile_pool(name="outp", bufs=2) as outp,
            tc.tile_pool(name="stats", bufs=4) as statp,
            tc.tile_pool(name="ps_s", bufs=3, space="PSUM") as ps_sp,
            tc.tile_pool(name="ps_t", bufs=3, space="PSUM") as ps_tp,
            tc.tile_pool(name="ps_o", bufs=1, space="PSUM") as ps_op,
        ):
            warm = constp.tile([P, 512], F32, name="warm")
            nc.gpsimd.memset(warm[:], 0.0)
            ident = constp.tile([P, P], AV_DT)
            make_identity(nc, ident[:])
            bias_res = None
            if causal:
                bias_res = constp.tile([P, 256], F32, name="bias_res")

            # K^T / V stay SBUF-resident; their loads are emitted inside the
            # q-tile loop in consumption order so q-tile 0's operands aren't
            # queued behind 16MB of K/V DMA.
            kt_sb = []   # per c-chunk: packed [P, 2*T] fp16 (hi|lo) or fp32
            for c in range(CCHUNKS):
                if QK_SPLIT:
                    kt_sb.append(kvp.tile([P, 2 * T], F16, tag=f"kt{c}",
                                          name=f"kt{c}"))
                else:
                    kt_sb.append(
                        kvp.tile([P, T], F32, tag=f"kt{c}", name=f"kt{c}"))
            v_sb = []
            for st in range(STILES):
                v_sb.append(kvp.tile([P, D], AV_DT, tag=f"v{st}", name=f"v{st}"))
            for w in range(8):
                ps_w = ps_sp.tile([P, 512], F32, tag="s", name="ps_w")
                nc.tensor.matmul(ps_w[:], warm[:, :P], warm[:],
                                 start=True, stop=True)

            kt_loaded = 0  # next 512-col chunk of kT to load
            v_loaded = 0   # next s-tile of V to load
            max_scols = max(sc for sc, _, _ in cfg)

            state = {}      # q-tile -> tensors produced by compute_a
            dma_state = {}  # q-tile -> qt tile in flight

            def dma_a(i):
                """Input DMAs for q-tile i (qt slab, new kT/V chunks)."""
                s_cols, b_off, b_cols = cfg[i]

                # one rearranged DMA per tensor for all 8 c-chunk slabs
                if QK_SPLIT:
                    qt_hl = qtp.tile([P, 2 * CCHUNKS * P], F16, tag="qt",
                                     name="qt_hl")
                    nc.sync.dma_start(
                        qt_hl.rearrange("p (h c j) -> p h c j", h=2, j=P),
                        qThl[:, :, i * P:(i + 1) * P].rearrange(
                            "(c p) h j -> p h c j", p=P))
                else:
                    qt_sb = qtp.tile([P, CCHUNKS * P], F32, tag="qt",
                                     name="qt_sb")
                    nc.sync.dma_start(
                        qt_sb.rearrange("p (c j) -> p c j", j=P),
                        qT[:, i * P:(i + 1) * P].rearrange("(c p) j -> p c j",
                                                           p=P))
                # kT column chunks first used by this q-tile (plus one chunk
                # of lookahead), then V s-tiles this q-tile newly needs.
                nonlocal kt_loaded, v_loaded
                want_kt = (min(s_cols, max_scols) + 511) // 512
                while kt_loaded < want_kt:
                    g = kt_loaded
                    for c in range(CCHUNKS):
                        if QK_SPLIT:
                            dst = kt_sb[c].rearrange("p (h t) -> p h t", h=2)
                            nc.sync.dma_start(
                                dst[:, :, g * 512:(g + 1) * 512],
                                kThl[c * P:(c + 1) * P, :,
                                     g * 512:(g + 1) * 512])
                        else:
                            nc.sync.dma_start(
                                kt_sb[c][:, g * 512:(g + 1) * 512],
                                kT[c * P:(c + 1) * P, g * 512:(g + 1) * 512])
                    kt_loaded += 1
                want_v = min(s_cols // P, STILES) if causal else STILES
                while v_loaded < want_v:
                    st = v_loaded
                    nc.sync.dma_start(v_sb[st][:], v[st * P:(st + 1) * P, :])
                    v_loaded += 1
                if causal:
                    if i == 0:
                        nc.sync.dma_start(bias_res[:], biasd[:])
                    bias_sb = bias_res
                else:
                    bias_sb = biasp.tile([P, b_cols], F32, tag="bias",
                                         name="bias_sb")
                    nc.sync.dma_start(bias_sb[:], biasd[i])
                dma_state[i] = (qt_hl if QK_SPLIT else qt_sb, bias_sb)

            def compute_a(i):
                """QK matmuls into per-group PSUM, copy to SBUF S, mask
                bias add, row-max stats."""
                s_cols, b_off, b_cols = cfg[i]
                ngroups = (s_cols + 511) // 512
                qt_t, bias_sb = dma_state.pop(i)
                if QK_SPLIT:
                    qt_hl = qt_t
                else:
                    qt_sb = qt_t

                s_sb = ssbp.tile([P, s_cols], F32, tag="s_sb", name="s_sb")
                pmax = statp.tile([P, ngroups], F32, tag="pmax", name="pmax")
                for g in range(ngroups):
                    g0 = g * 512
                    gw = min(512, s_cols - g0)
                    ps = ps_sp.tile([P, 512], F32, tag="s", name="ps_g")
                    for c in range(CCHUNKS):
                        if QK_SPLIT:
                            QH = CCHUNKS * P  # lo-plane column offset in qt_hl
                            terms = [(0, 0), (0, T), (QH, 0)]
                            for ti, (qo, ko) in enumerate(terms):
                                nc.tensor.matmul(
                                    ps[:, :gw],
                                    qt_hl[:, qo + c * P:qo + (c + 1) * P],
                                    kt_sb[c][:, ko + g0:ko + g0 + gw],
                                    start=(c == 0 and ti == 0),
                                    stop=(c == CCHUNKS - 1 and ti == 2))
                        else:
                            nc.tensor.matmul(
                                ps[:, :gw],
                                qt_sb[:, c * P:(c + 1) * P],
                                kt_sb[c][:, g0:g0 + gw],
                                start=(c == 0), stop=(c == CCHUNKS - 1))
                    # PSUM -> SBUF: plain copy outside the mask band (ACT),
                    # fused bias-add inside it (DVE).
                    lo = max(g0, b_off)
                    hi = min(g0 + gw, b_off + b_cols)
                    if lo < hi:
                        if lo > g0:
                            nc.scalar.copy(s_sb[:, g0:lo], ps[:, :lo - g0])
                        nc.vector.tensor_add(
                            s_sb[:, lo:hi], ps[:, lo - g0:hi - g0],
                            bias_sb[:, lo - b_off:hi - b_off])
                        if hi < g0 + gw:
                            nc.scalar.copy(s_sb[:, hi:g0 + gw],
                                           ps[:, hi - g0:gw])
                    else:
                        nc.scalar.copy(s_sb[:, g0:g0 + gw], ps[:, :gw])
                    nc.vector.reduce_max(pmax[:, g:g + 1], s_sb[:, g0:g0 + gw],
                                         axis=AX)
                negm = statp.tile([P, 1], F32, tag="negm", name="negm")
                nc.vector.reduce_max(negm[:], pmax[:, :ngroups], axis=AX,
                                     negate=True)
                negm32 = statp.tile([P, 1], F32, tag="negm32", name="negm32")
                nc.vector.tensor_scalar_mul(negm32[:], negm[:], 32.0)
                state[i] = (s_sb, negm32)

            def stage_b(i):
                """exp + row-sum, P^T transposes, AV accumulation, 1/sum
                scale, output DMA."""
                s_cols, _, _ = cfg[i]
                stiles = s_cols // P
                ngroups = (s_cols + 511) // 512
                s_sb, negm32 = state.pop(i)

                p_sb = pp.tile([P, s_cols], AV_DT, tag="p", name="p_sb")
                gsum = statp.tile([P, ngroups], F32, tag="gsum", name="gsum")
                for g in range(ngroups):
                    g0 = g * 512
                    gw = min(512, s_cols - g0)
                    nc.scalar.activation(
                        p_sb[:, g0:g0 + gw], s_sb[:, g0:g0 + gw], EXP,
                        bias=negm32[:], scale=32.0,
                        accum_out=gsum[:, g:g + 1])
                rsum = statp.tile([P, 1], F32, tag="rsum", name="rsum")
                nc.vector.reduce_sum(rsum[:], gsum[:, :ngroups], axis=AX)
                rinv = statp.tile([P, 1], F32, tag="rinv", name="rinv")
                nc.vector.reciprocal(rinv[:], rsum[:])

                ps_o = ps_op.tile([P, D], F32, tag="o", name="ps_o")
                for st in range(stiles):
                    ps_t = ps_tp.tile([P, P], AV_DT, tag="t", name="ps_t")
                    nc.tensor.transpose(ps_t[:], p_sb[:, st * P:(st + 1) * P],
                                        ident[:])
                    pt_sb = ptp.tile([P, P], AV_DT, tag="pt", name="pt_sb")
                    nc.vector.tensor_copy(pt_sb[:], ps_t[:])
                    for dh in range(2):
                        nc.tensor.matmul(
                            ps_o[:, dh * 512:(dh + 1) * 512],
                            pt_sb[:],
                            v_sb[st][:, dh * 512:(dh + 1) * 512],
                            start=(st == 0), stop=(st == stiles - 1))
                o_sb = outp.tile([P, D], F32, tag="o_sb", name="o_sb")
                nc.vector.tensor_scalar_mul(o_sb[:], ps_o[:], rinv[:])
                nc.sync.dma_start(out[i * P:(i + 1) * P, :], o_sb[:])

            # Software pipeline: QK of one tile runs (on PE) while the
            # previous tile does softmax/exp on ACT/DVE, so PE never waits
            # on the softmax.  Tile 2 is moved last so the un-overlapped
            # final B stage is a small one (6 key blocks instead of 16).
            order = [0, 1, 3, 4, 5, 7, 6, 2]
            dma_a(order[0])
            for idx in range(len(order) + 1):
                if idx < len(order):
                    # issue the NEXT tile's DMAs first so its operands are
                    # in flight while this tile's QK runs
                    if idx + 1 < len(order):
                        dma_a(order[idx + 1])
                    compute_a(order[idx])
                if idx > 0:
                    stage_b(order[idx - 1])

    nc.compile()
    return nc


def _rows(causal: bool, p: int) -> np.ndarray:
    if causal:
        return np.concatenate(
            [256 * i + 2 * np.arange(P) + p for i in range(NQT)])
    return p * (NQT * P) + np.arange(NQT * P)


def _get(causal: bool):
    if causal not in _cache:
        _cache[causal] = _build(causal)
    return _cache[causal]


def kernel(query, key, value, mask):
    query = np.asarray(query, dtype=np.float32)
    key = np.asarray(key, dtype=np.float32)
    value = np.asarray(value, dtype=np.float32)
    mask = np.asarray(mask, dtype=np.float32)

    causal = bool(
        np.array_equal(mask, np.triu(np.ones((T, T), np.float32), k=1)))
    nc = _get(causal)
    cfg = _tile_cfg(causal)
    # bias folded pre-scale: 32*(S + mask*NEG/32) == 32*S + mask*NEG exactly
    mask_scaled = mask * np.float32(NEG / 32.0)

    def hilo_packed(x):
        # [D, n] fp32 -> [D, 2, n] fp16 with planes (hi, lo)
        hi = x.astype(np.float16)
        lo = (x - hi.astype(np.float32)).astype(np.float16)
        return np.ascontiguousarray(np.stack([hi, lo], axis=1))

    if QK_SPLIT:
        kT_hl = [hilo_packed(np.ascontiguousarray(key[b].T))
                 for b in range(B)]
    else:
        kTs = [np.ascontiguousarray(key[b].T) for b in range(B)]
    in_maps = []
    rows_by_core = []
    for c in range(NCORES):
        b, p = c // 2, c % 2
        rows = _rows(causal, p)
        rows_by_core.append((b, rows))
        qT_c = np.ascontiguousarray(query[b][rows].T)
        if causal:
            _, boff, bcols = cfg[0]
            bias_c = mask_scaled[rows[0:P], boff:boff + bcols]
        else:
            bias_c = np.stack([
                mask_scaled[rows[i * P:(i + 1) * P], boff:boff + bcols]
                for i, (_, boff, bcols) in enumerate(cfg)])
        im = {
            "v": np.ascontiguousarray(value[b]).astype(
                np.float16 if AV_DT == F16 else np.float32),
            "bias": np.ascontiguousarray(bias_c),
        }
        if QK_SPLIT:
            im["qThl"] = hilo_packed(qT_c)
            im["kThl"] = kT_hl[b]
        else:
            im["qT"] = qT_c
            im["kT"] = kTs[b]
        in_maps.append(im)

    res = run_bass_kernel_spmd(nc, in_maps, core_ids=list(range(NCORES)))

    outp = np.empty((B, T, D), dtype=np.float32)
    for c in range(NCORES):
        b, rows = rows_by_core[c]
        outp[b][rows] = res.results[c]["out"]
    return outp


# revision 30
# speedup vs baseline: 1.0047x; 1.0047x over previous
"""Causal single-head attention (B=4, T=2048, D=1024, fp32) on 8 trn2 cores.

Sharding: each core takes one (batch, parity) pair: batch b = core//2,
parity p = core%2.  Within its batch, a core owns the query rows
{256*i + 2*j + p : i in 0..7, j in 0..127} -- i.e. 8 query tiles of 128
rows, where tile i holds every-other row of the global row range
[256*i, 256*(i+1)).  With a causal mask, tile i only needs keys
[0, 256*(i+1)), so the per-tile key length (2*(i+1) blocks of 128) is
identical for both parities -> one SPMD program, perfectly load-balanced,
and ~1.8x less matmul work than dense.

Per q-tile pipeline (per core):
  S = Q_tile @ K^T (PE).  Q and K are split host-side into fp16 hi/lo
     pairs and S is computed as qh@kh + qh@kl + ql@kh (3 fp16 passes at
     1 PE cycle/row ~= fp32 precision, vs native fp32's 4 cycles/row;
     the PE multiplies fp16 subnormals exactly and the dropped ql@kl
     term is below fp32 accumulation noise), accumulated in fp32 PSUM
     over 8 c-chunks.
  PSUM -> SBUF copy (ACT) with mask-bias add on the diagonal band (DVE,
     from the real mask input), group-wise row maxes pipelined behind
     the matmuls (DVE).
  P = exp(32*S - 32*max) (ACT, fp16 out, row-sums via accum_out)
  P^T per 128-block (PE transpose via identity) -> O += P^T.T @ V
     (PE, fp16 operands, fp32 PSUM accumulation)
  O *= 1/rowsum (DVE), DMA out.
Stage B of tile i runs on ACT/DVE while stage A (QK) of the next tile
runs on the PE, so the PE never waits on the softmax.  Warm-up matmuls
on a zeroed tile cover the initial DMA prologue and keep the PE's HAM
clock gate at full rate.

If the mask input is NOT exactly the causal triu mask, falls back to a
dense variant of the same program (all 16 key blocks per q-tile, full
mask bias applied) which is correct for any additive {0,1} mask.
"""

import os

import numpy as np

import concourse.mybir as mybir
import concourse.tile as tile
from concourse import bacc
from concourse.bass_utils import run_bass_kernel_spmd
from concourse.masks import make_identity

B, T, D = 4, 2048, 1024
NEG = -1000000000.0
P = 128          # partitions
NCORES = 8
NQT = 8          # q-tiles of 128 rows per core
CCHUNKS = D // P  # 8 contraction chunks
STILES = T // P   # 16 key tiles per batch
F32 = mybir.dt.float32
F16 = mybir.dt.float16

# AV (P @ V) operand dtype: fp16 runs the PE at 1 cycle/row vs fp32's 4.
# P in [0,1] and V ~ N(0,1) both fit fp16 with ~2^-11 relative rounding.
AV_DT = F32 if os.environ.get("KERNEL_AV_F32", "0") == "1" else F16
# QK path: fp16 hi/lo split, S = qh@kh + qh@kl + ql@kh (3 passes at
# 1 cycle/row) instead of native fp32 (4 cycles/row).  The split keeps
# ~22 mantissa bits; the PE multiplies fp16 subnormals exactly (verified
# on HW), and the dropped ql@kl term is below fp32 accumulation noise.
QK_SPLIT = os.environ.get("KERNEL_QK_F32", "0") != "1"
_cache = {}


def _tile_cfg(causal: bool):
    """Per-q-tile (s_cols, bias_off, bias_cols)."""
    if causal:
        return [(256 * (i + 1), 256 * i, 256) for i in range(NQT)]
    return [(T, 0, T) for _ in range(NQT)]


def _build(causal: bool):
    cfg = _tile_cfg(causal)
    bias_cols = cfg[0][2]

    nc = bacc.Bacc("TRN2", target_bir_lowering=False, debug=False,
                   num_devices=NCORES)
    if QK_SPLIT:
        qThl = nc.declare_dram_parameter("qThl", [D, 2, NQT * P], F16,
                                         isOutput=False)
        kThl = nc.declare_dram_parameter("kThl", [D, 2, T], F16,
                                         isOutput=False)
    else:
        qT = nc.declare_dram_parameter("qT", [D, NQT * P], F32, isOutput=False)
        kT = nc.declare_dram_parameter("kT", [D, T], F32, isOutput=False)
    v = nc.declare_dram_parameter("v", [T, D], AV_DT, isOutput=False)
    # For the causal path the diagonal-band bias block is identical for
    # every q-tile (band entry (j, u) is masked iff u > 2j + parity), so a
    # single [P, 256] input suffices; the dense path keeps per-tile rows.
    if causal:
        biasd = nc.declare_dram_parameter("bias", [P, bias_cols], F32,
                                          isOutput=False)
    else:
        biasd = nc.declare_dram_parameter("bias", [NQT, P, bias_cols], F32,
                                          isOutput=False)
    out = nc.declare_dram_parameter("out", [NQT * P, D], F32, isOutput=True)

    AX = mybir.AxisListType.X
    EXP = mybir.ActivationFunctionType.Exp

    with tile.TileContext(nc) as tc:
        with (
            tc.tile_pool(name="const", bufs=1) as constp,
            tc.tile_pool(name="kv", bufs=1) as kvp,
            tc.tile_pool(name="qt", bufs=2) as qtp,
            tc.tile_pool(name="biasp", bufs=2) as biasp,
            tc.tile_pool(name="pp", bufs=2) as pp,
            tc.tile_pool(name="ssb", bufs=2) as ssbp,
            tc.tile_pool(name="ptp", bufs=3) as ptp,
            tc.tile_pool(name="outp", bufs=2) as outp,
            tc.tile_pool(name="stats", bufs=4) as statp,
            tc.tile_pool(name="ps_s", bufs=3, space="PSUM") as ps_sp,
            tc.tile_pool(name="ps_t", bufs=3, space="PSUM") as ps_tp,
            tc.tile_pool(name="ps_o", bufs=1, space="PSUM") as ps_op,
        ):
            warm = constp.tile([P, 512], F32, name="warm")
            nc.gpsimd.memset(warm[:], 0.0)
            ident = constp.tile([P, P], AV_DT)
            make_identity(nc, ident[:])
            bias_res = None
            if causal:
                bias_res = constp.tile([P, 256], F32, name="bias_res")

            # K^T / V stay SBUF-resident; their loads are emitted inside the
            # q-tile loop in consumption order so q-tile 0's operands aren't
            # queued behind 16MB of K/V DMA.
            kt_sb = []   # per c-chunk: packed [P, 2*T] fp16 (hi|lo) or fp32
            for c in range(CCHUNKS):
                if QK_SPLIT:
                    kt_sb.append(kvp.tile([P, 2 * T], F16, tag=f"kt{c}",
                                          name=f"kt{c}"))
                else:
                    kt_sb.append(
                        kvp.tile([P, T], F32, tag=f"kt{c}", name=f"kt{c}"))
            v_sb = []
            for st in range(STILES):
                v_sb.append(kvp.tile([P, D], AV_DT, tag=f"v{st}", name=f"v{st}"))
            for w in range(8):
                ps_w = ps_sp.tile([P, 512], F32, tag="s", name="ps_w")
                nc.tensor.matmul(ps_w[:], warm[:, :P], warm[:],
                                 start=True, stop=True)

            kt_loaded = 0  # next 512-col chunk of kT to load
            v_loaded = 0   # next s-tile of V to load
            max_scols = max(sc for sc, _, _ in cfg)

            state = {}      # q-tile -> tensors produced by compute_a
            dma_state = {}  # q-tile -> qt tile in flight

            def dma_a(i):
                """Input DMAs for q-tile i (qt slab, new kT/V chunks)."""
                s_cols, b_off, b_cols = cfg[i]

                # one rearranged DMA per tensor for all 8 c-chunk slabs
                if QK_SPLIT:
                    qt_hl = qtp.tile([P, 2 * CCHUNKS * P], F16, tag="qt",
                                     name="qt_hl")
                    nc.sync.dma_start(
                        qt_hl.rearrange("p (h c j) -> p h c j", h=2, j=P),
                        qThl[:, :, i * P:(i + 1) * P].rearrange(
                            "(c p) h j -> p h c j", p=P))
                else:
                    qt_sb = qtp.tile([P, CCHUNKS * P], F32, tag="qt",
                                     name="qt_sb")
                    nc.sync.dma_start(
                        qt_sb.rearrange("p (c j) -> p c j", j=P),
                        qT[:, i * P:(i + 1) * P].rearrange("(c p) j -> p c j",
                                                           p=P))
                # kT column chunks first used by this q-tile (plus one chunk
                # of lookahead), then V s-tiles this q-tile newly needs.
                nonlocal kt_loaded, v_loaded
                want_kt = (min(s_cols, max_scols) + 511) // 512
                while kt_loaded < want_kt:
                    g = kt_loaded
                    for c in range(CCHUNKS):
                        if QK_SPLIT:
                            dst = kt_sb[c].rearrange("p (h t) -> p h t", h=2)
                            nc.sync.dma_start(
                                dst[:, :, g * 512:(g + 1) * 512],
                                kThl[c * P:(c + 1) * P, :,
                                     g * 512:(g + 1) * 512])
                        else:
                            nc.sync.dma_start(
                                kt_sb[c][:, g * 512:(g + 1) * 512],
                                kT[c * P:(c + 1) * P, g * 512:(g + 1) * 512])
                    kt_loaded += 1
                want_v = min(s_cols // P, STILES) if causal else STILES
                while v_loaded < want_v:
                    st = v_loaded
                    nc.sync.dma_start(v_sb[st][:], v[st * P:(st + 1) * P, :])
                    v_loaded += 1
                if causal:
                    if i == 0:
                        nc.sync.dma_start(bias_res[:], biasd[:])
                    bias_sb = bias_res
                else:
                    bias_sb = biasp.tile([P, b_cols], F32, tag="bias",
                                         name="bias_sb")
                    nc.sync.dma_start(bias_sb[:], biasd[i])
                dma_state[i] = (qt_hl if QK_SPLIT else qt_sb, bias_sb)

            def compute_a(i):
                """QK matmuls into per-group PSUM, copy to SBUF S, mask
                bias add, row-max stats."""
                s_cols, b_off, b_cols = cfg[i]
                ngroups = (s_cols + 511) // 512
                qt_t, bias_sb = dma_state.pop(i)
                if QK_SPLIT:
                    qt_hl = qt_t
                else:
                    qt_sb = qt_t

                s_sb = ssbp.tile([P, s_cols], F32, tag="s_sb", name="s_sb")
                pmax = statp.tile([P, ngroups], F32, tag="pmax", name="pmax")
                for g in range(ngroups):
                    g0 = g * 512
                    gw = min(512, s_cols - g0)
                    ps = ps_sp.tile([P, 512], F32, tag="s", name="ps_g")
                    for c in range(CCHUNKS):
                        if QK_SPLIT:
                            QH = CCHUNKS * P  # lo-plane column offset in qt_hl
                            terms = [(0, 0), (0, T), (QH, 0)]
                            for ti, (qo, ko) in enumerate(terms):
                                nc.tensor.matmul(
                                    ps[:, :gw],
                                    qt_hl[:, qo + c * P:qo + (c + 1) * P],
                                    kt_sb[c][:, ko + g0:ko + g0 + gw],
                                    start=(c == 0 and ti == 0),
                                    stop=(c == CCHUNKS - 1 and ti == 2))
                        else:
                            nc.tensor.matmul(
                                ps[:, :gw],
                                qt_sb[:, c * P:(c + 1) * P],
                                kt_sb[c][:, g0:g0 + gw],
                                start=(c == 0), stop=(c == CCHUNKS - 1))
                    # PSUM -> SBUF: plain copy outside the mask band (ACT),
                    # fused bias-add inside it (DVE).
                    lo = max(g0, b_off)
                    hi = min(g0 + gw, b_off + b_cols)
                    if lo < hi:
                        if lo > g0:
                            nc.scalar.copy(s_sb[:, g0:lo], ps[:, :lo - g0])
                        nc.vector.tensor_add(
                            s_sb[:, lo:hi], ps[:, lo - g0:hi - g0],
                            bias_sb[:, lo - b_off:hi - b_off])
                        if hi < g0 + gw:
                            nc.scalar.copy(s_sb[:, hi:g0 + gw],
                                           ps[:, hi - g0:gw])
                    else:
                        nc.scalar.copy(s_sb[:, g0:g0 + gw], ps[:, :gw])
                    nc.vector.reduce_max(pmax[:, g:g + 1], s_sb[:, g0:g0 + gw],
                                         axis=AX)
                negm = statp.tile([P, 1], F32, tag="negm", name="negm")
                nc.vector.reduce_max(negm[:], pmax[:, :ngroups], axis=AX,
                                     negate=True)
                negm32 = statp.tile([P, 1], F32, tag="negm32", name="negm32")
                nc.vector.tensor_scalar_mul(negm32[:], negm[:], 32.0)
                state[i] = (s_sb, negm32)

            def stage_b(i):
                """exp + row-sum, P^T transposes, AV accumulation, 1/sum
                scale, output DMA."""
                s_cols, _, _ = cfg[i]
                stiles = s_cols // P
                ngroups = (s_cols + 511) // 512
                s_sb, negm32 = state.pop(i)

                p_sb = pp.tile([P, s_cols], AV_DT, tag="p", name="p_sb")
                gsum = statp.tile([P, ngroups], F32, tag="gsum", name="gsum")
                for g in range(ngroups):
                    g0 = g * 512
                    gw = min(512, s_cols - g0)
                    nc.scalar.activation(
                        p_sb[:, g0:g0 + gw], s_sb[:, g0:g0 + gw], EXP,
                        bias=negm32[:], scale=32.0,
                        accum_out=gsum[:, g:g + 1])
                rsum = statp.tile([P, 1], F32, tag="rsum", name="rsum")
                nc.vector.reduce_sum(rsum[:], gsum[:, :ngroups], axis=AX)
                rinv = statp.tile([P, 1], F32, tag="rinv", name="rinv")
                nc.vector.reciprocal(rinv[:], rsum[:])

                ps_o = ps_op.tile([P, D], F32, tag="o", name="ps_o")
                for st in range(stiles):
                    ps_t = ps_tp.tile([P, P], AV_DT, tag="t", name="ps_t")
                    nc.tensor.transpose(ps_t[:], p_sb[:, st * P:(st + 1) * P],
                                        ident[:])
                    pt_sb = ptp.tile([P, P], AV_DT, tag="pt", name="pt_sb")
                    nc.vector.tensor_copy(pt_sb[:], ps_t[:])
                    for dh in range(2):
                        nc.tensor.matmul(
                            ps_o[:, dh * 512:(dh + 1) * 512],
                            pt_sb[:],
                            v_sb[st][:, dh * 512:(dh + 1) * 512],
                            start=(st == 0), stop=(st == stiles - 1))
                o_sb = outp.tile([P, D], F32, tag="o_sb", name="o_sb")
                # scale + store per d-half so the first half's DMA overlaps
                # the second half's scale (shortens the kernel tail)
                for dh in range(2):
                    dsl = slice(dh * 512, (dh + 1) * 512)
                    nc.vector.tensor_scalar_mul(o_sb[:, dsl], ps_o[:, dsl],
                                                rinv[:])
                    nc.sync.dma_start(out[i * P:(i + 1) * P, dsl],
                                      o_sb[:, dsl])

            # Software pipeline: QK of one tile runs (on PE) while the
            # previous tile does softmax/exp on ACT/DVE, so PE never waits
            # on the softmax.  Tile 2 is moved last so the un-overlapped
            # final B stage is a small one (6 key blocks instead of 16).
            order = [0, 1, 3, 4, 5, 7, 6, 2]
            dma_a(order[0])
            for idx in range(len(order) + 1):
                if idx < len(order):
                    # issue the NEXT tile's DMAs first so its operands are
                    # in flight while this tile's QK runs
                    if idx + 1 < len(order):
                        dma_a(order[idx + 1])
                    compute_a(order[idx])
                if idx > 0:
                    stage_b(order[idx - 1])

    nc.compile()
    return nc


def _rows(causal: bool, p: int) -> np.ndarray:
    if causal:
        return np.concatenate(
            [256 * i + 2 * np.arange(P) + p for i in range(NQT)])
    return p * (NQT * P) + np.arange(NQT * P)


def _get(causal: bool):
    if causal not in _cache:
        _cache[causal] = _build(causal)
    return _cache[causal]


def kernel(query, key, value, mask):
    query = np.asarray(query, dtype=np.float32)
    key = np.asarray(key, dtype=np.float32)
    value = np.asarray(value, dtype=np.float32)
    mask = np.asarray(mask, dtype=np.float32)

    causal = bool(
        np.array_equal(mask, np.triu(np.ones((T, T), np.float32), k=1)))
    nc = _get(causal)
    cfg = _tile_cfg(causal)
    # bias folded pre-scale: 32*(S + mask*NEG/32) == 32*S + mask*NEG exactly
    mask_scaled = mask * np.float32(NEG / 32.0)

    def hilo_packed(x):
        # [D, n] fp32 -> [D, 2, n] fp16 with planes (hi, lo)
        hi = x.astype(np.float16)
        lo = (x - hi.astype(np.float32)).astype(np.float16)
        return np.ascontiguousarray(np.stack([hi, lo], axis=1))

    if QK_SPLIT:
        kT_hl = [hilo_packed(np.ascontiguousarray(key[b].T))
                 for b in range(B)]
    else:
        kTs = [np.ascontiguousarray(key[b].T) for b in range(B)]
    in_maps = []
    rows_by_core = []
    for c in range(NCORES):
        b, p = c // 2, c % 2
        rows = _rows(causal, p)
        rows_by_core.append((b, rows))
        qT_c = np.ascontiguousarray(query[b][rows].T)
        if causal:
            _, boff, bcols = cfg[0]
            bias_c = mask_scaled[rows[0:P], boff:boff + bcols]
        else:
            bias_c = np.stack([
                mask_scaled[rows[i * P:(i + 1) * P], boff:boff + bcols]
                for i, (_, boff, bcols) in enumerate(cfg)])
        im = {
            "v": np.ascontiguousarray(value[b]).astype(
                np.float16 if AV_DT == F16 else np.float32),
            "bias": np.ascontiguousarray(bias_c),
        }
        if QK_SPLIT:
            im["qThl"] = hilo_packed(qT_c)
            im["kThl"] = kT_hl[b]
        else:
            im["qT"] = qT_c
            im["kT"] = kTs[b]
        in_maps.append(im)

    res = run_bass_kernel_spmd(nc, in_maps, core_ids=list(range(NCORES)))

    outp = np.empty((B, T, D), dtype=np.float32)
    for c in range(NCORES):
        b, rows = rows_by_core[c]
        outp[b][rows] = res.results[c]["out"]
    return outp


# revision 34
# speedup vs baseline: 1.0122x; 1.0075x over previous
"""Causal single-head attention (B=4, T=2048, D=1024, fp32) on 8 trn2 cores.

Sharding: each core takes one (batch, parity) pair: batch b = core//2,
parity p = core%2.  Within its batch, a core owns the query rows
{256*i + 2*j + p : i in 0..7, j in 0..127} -- i.e. 8 query tiles of 128
rows, where tile i holds every-other row of the global row range
[256*i, 256*(i+1)).  With a causal mask, tile i only needs keys
[0, 256*(i+1)), so the per-tile key length (2*(i+1) blocks of 128) is
identical for both parities -> one SPMD program, perfectly load-balanced,
and ~1.8x less matmul work than dense.

Per q-tile pipeline (per core):
  S = Q_tile @ K^T (PE).  Q and K are split host-side into fp16 hi/lo
     pairs and S is computed as qh@kh + qh@kl + ql@kh (3 fp16 passes at
     1 PE cycle/row ~= fp32 precision, vs native fp32's 4 cycles/row;
     the PE multiplies fp16 subnormals exactly and the dropped ql@kl
     term is below fp32 accumulation noise), accumulated in fp32 PSUM
     over 8 c-chunks.
  PSUM -> SBUF copy (ACT) with mask-bias add on the diagonal band (DVE,
     from the real mask input), group-wise row maxes pipelined behind
     the matmuls (DVE).
  P = exp(32*S - 32*max) (ACT, fp16 out, row-sums via accum_out)
  P^T per 128-block (PE transpose via identity) -> O += P^T.T @ V
     (PE, fp16 operands, fp32 PSUM accumulation)
  O *= 1/rowsum (DVE), DMA out.
Stage B of tile i runs on ACT/DVE while stage A (QK) of the next tile
runs on the PE, so the PE never waits on the softmax.  Warm-up matmuls
on a zeroed tile cover the initial DMA prologue and keep the PE's HAM
clock gate at full rate.

If the mask input is NOT exactly the causal triu mask, falls back to a
dense variant of the same program (all 16 key blocks per q-tile, full
mask bias applied) which is correct for any additive {0,1} mask.
"""

import os

import numpy as np

import concourse.mybir as mybir
import concourse.tile as tile
from concourse import bacc
from concourse.bass_utils import run_bass_kernel_spmd
from concourse.masks import make_identity

B, T, D = 4, 2048, 1024
NEG = -1000000000.0
P = 128          # partitions
NCORES = 8
NQT = 8          # q-tiles of 128 rows per core
CCHUNKS = D // P  # 8 contraction chunks
STILES = T // P   # 16 key tiles per batch
F32 = mybir.dt.float32
F16 = mybir.dt.float16

# AV (P @ V) operand dtype: fp16 runs the PE at 1 cycle/row vs fp32's 4.
# P in [0,1] and V ~ N(0,1) both fit fp16 with ~2^-11 relative rounding.
AV_DT = F32 if os.environ.get("KERNEL_AV_F32", "0") == "1" else F16
# QK path: fp16 hi/lo split, S = qh@kh + qh@kl + ql@kh (3 passes at
# 1 cycle/row) instead of native fp32 (4 cycles/row).  The split keeps
# ~22 mantissa bits; the PE multiplies fp16 subnormals exactly (verified
# on HW), and the dropped ql@kl term is below fp32 accumulation noise.
QK_SPLIT = os.environ.get("KERNEL_QK_F32", "0") != "1"
_cache = {}


def _tile_cfg(causal: bool):
    """Per-q-tile (s_cols, bias_off, bias_cols)."""
    if causal:
        return [(256 * (i + 1), 256 * i, 256) for i in range(NQT)]
    return [(T, 0, T) for _ in range(NQT)]


def _build(causal: bool):
    cfg = _tile_cfg(causal)
    bias_cols = cfg[0][2]

    nc = bacc.Bacc("TRN2", target_bir_lowering=False, debug=False,
                   num_devices=NCORES)
    if QK_SPLIT:
        qThl = nc.declare_dram_parameter("qThl", [D, 2, NQT * P], F16,
                                         isOutput=False)
        kThl = nc.declare_dram_parameter("kThl", [D, 2, T], F16,
                                         isOutput=False)
    else:
        qT = nc.declare_dram_parameter("qT", [D, NQT * P], F32, isOutput=False)
        kT = nc.declare_dram_parameter("kT", [D, T], F32, isOutput=False)
    v = nc.declare_dram_parameter("v", [T, D], AV_DT, isOutput=False)
    # For the causal path the diagonal-band bias block is identical for
    # every q-tile (band entry (j, u) is masked iff u > 2j + parity), so a
    # single [P, 256] input suffices; the dense path keeps per-tile rows.
    if causal:
        biasd = nc.declare_dram_parameter("bias", [P, bias_cols], F32,
                                          isOutput=False)
    else:
        biasd = nc.declare_dram_parameter("bias", [NQT, P, bias_cols], F32,
                                          isOutput=False)
    out = nc.declare_dram_parameter("out", [NQT * P, D], F32, isOutput=True)

    AX = mybir.AxisListType.X
    EXP = mybir.ActivationFunctionType.Exp

    with tile.TileContext(nc) as tc:
        with (
            tc.tile_pool(name="const", bufs=1) as constp,
            tc.tile_pool(name="kv", bufs=1) as kvp,
            tc.tile_pool(name="qt", bufs=2) as qtp,
            tc.tile_pool(name="biasp", bufs=2) as biasp,
            tc.tile_pool(name="pp", bufs=2) as pp,
            tc.tile_pool(name="ssb", bufs=2) as ssbp,
            tc.tile_pool(name="ptp", bufs=3) as ptp,
            tc.tile_pool(name="outp", bufs=2) as outp,
            tc.tile_pool(name="stats", bufs=4) as statp,
            tc.tile_pool(name="ps_s", bufs=3, space="PSUM") as ps_sp,
            tc.tile_pool(name="ps_t", bufs=3, space="PSUM") as ps_tp,
            tc.tile_pool(name="ps_o", bufs=1, space="PSUM") as ps_op,
        ):
            warm = constp.tile([P, 256], F32, name="warm")
            nc.gpsimd.memset(warm[:], 0.0)
            ident = constp.tile([P, P], AV_DT)
            make_identity(nc, ident[:])
            bias_res = None
            if causal:
                bias_res = constp.tile([P, 256], F32, name="bias_res")

            # K^T / V stay SBUF-resident; their loads are emitted inside the
            # q-tile loop in consumption order so q-tile 0's operands aren't
            # queued behind 16MB of K/V DMA.
            kt_sb = []   # per c-chunk: packed [P, 2*T] fp16 (hi|lo) or fp32
            for c in range(CCHUNKS):
                if QK_SPLIT:
                    kt_sb.append(kvp.tile([P, 2 * T], F16, tag=f"kt{c}",
                                          name=f"kt{c}"))
                else:
                    kt_sb.append(
                        kvp.tile([P, T], F32, tag=f"kt{c}", name=f"kt{c}"))
            v_sb = []
            for st in range(STILES):
                v_sb.append(kvp.tile([P, D], AV_DT, tag=f"v{st}", name=f"v{st}"))
            for w in range(16):
                ps_w = ps_sp.tile([P, 512], F32, tag="s", name="ps_w")
                nc.tensor.matmul(ps_w[:, :256], warm[:, :P], warm[:],
                                 start=True, stop=True)

            kt_loaded = 0  # next 512-col chunk of kT to load
            v_loaded = 0   # next s-tile of V to load
            max_scols = max(sc for sc, _, _ in cfg)

            state = {}      # q-tile -> tensors produced by compute_a
            dma_state = {}  # q-tile -> qt tile in flight

            def dma_a(i):
                """Input DMAs for q-tile i (qt slab, new kT/V chunks)."""
                s_cols, b_off, b_cols = cfg[i]

                # one rearranged DMA per tensor for all 8 c-chunk slabs
                if QK_SPLIT:
                    qt_hl = qtp.tile([P, 2 * CCHUNKS * P], F16, tag="qt",
                                     name="qt_hl")
                    nc.sync.dma_start(
                        qt_hl.rearrange("p (h c j) -> p h c j", h=2, j=P),
                        qThl[:, :, i * P:(i + 1) * P].rearrange(
                            "(c p) h j -> p h c j", p=P))
                else:
                    qt_sb = qtp.tile([P, CCHUNKS * P], F32, tag="qt",
                                     name="qt_sb")
                    nc.sync.dma_start(
                        qt_sb.rearrange("p (c j) -> p c j", j=P),
                        qT[:, i * P:(i + 1) * P].rearrange("(c p) j -> p c j",
                                                           p=P))
                # kT column chunks first used by this q-tile (plus one chunk
                # of lookahead), then V s-tiles this q-tile newly needs.
                nonlocal kt_loaded, v_loaded
                want_kt = (min(s_cols, max_scols) + 511) // 512
                while kt_loaded < want_kt:
                    g = kt_loaded
                    for c in range(CCHUNKS):
                        if QK_SPLIT:
                            dst = kt_sb[c].rearrange("p (h t) -> p h t", h=2)
                            nc.sync.dma_start(
                                dst[:, :, g * 512:(g + 1) * 512],
                                kThl[c * P:(c + 1) * P, :,
                                     g * 512:(g + 1) * 512])
                        else:
                            nc.sync.dma_start(
                                kt_sb[c][:, g * 512:(g + 1) * 512],
                                kT[c * P:(c + 1) * P, g * 512:(g + 1) * 512])
                    kt_loaded += 1
                want_v = min(s_cols // P, STILES) if causal else STILES
                while v_loaded < want_v:
                    st = v_loaded
                    nc.sync.dma_start(v_sb[st][:], v[st * P:(st + 1) * P, :])
                    v_loaded += 1
                if causal:
                    if i == 0:
                        nc.sync.dma_start(bias_res[:], biasd[:])
                    bias_sb = bias_res
                else:
                    bias_sb = biasp.tile([P, b_cols], F32, tag="bias",
                                         name="bias_sb")
                    nc.sync.dma_start(bias_sb[:], biasd[i])
                dma_state[i] = (qt_hl if QK_SPLIT else qt_sb, bias_sb)

            def compute_a(i):
                """QK matmuls into per-group PSUM, copy to SBUF S, mask
                bias add, row-max stats."""
                s_cols, b_off, b_cols = cfg[i]
                ngroups = (s_cols + 511) // 512
                qt_t, bias_sb = dma_state.pop(i)
                if QK_SPLIT:
                    qt_hl = qt_t
                else:
                    qt_sb = qt_t

                s_sb = ssbp.tile([P, s_cols], F32, tag="s_sb", name="s_sb")
                pmax = statp.tile([P, ngroups], F32, tag="pmax", name="pmax")
                for g in range(ngroups):
                    g0 = g * 512
                    gw = min(512, s_cols - g0)
                    ps = ps_sp.tile([P, 512], F32, tag="s", name="ps_g")
                    for c in range(CCHUNKS):
                        if QK_SPLIT:
                            QH = CCHUNKS * P  # lo-plane column offset in qt_hl
                            terms = [(0, 0), (0, T), (QH, 0)]
                            for ti, (qo, ko) in enumerate(terms):
                                nc.tensor.matmul(
                                    ps[:, :gw],
                                    qt_hl[:, qo + c * P:qo + (c + 1) * P],
                                    kt_sb[c][:, ko + g0:ko + g0 + gw],
                                    start=(c == 0 and ti == 0),
                                    stop=(c == CCHUNKS - 1 and ti == 2))
                        else:
                            nc.tensor.matmul(
                                ps[:, :gw],
                                qt_sb[:, c * P:(c + 1) * P],
                                kt_sb[c][:, g0:g0 + gw],
                                start=(c == 0), stop=(c == CCHUNKS - 1))
                    # PSUM -> SBUF: plain copy outside the mask band (ACT),
                    # fused bias-add inside it (DVE).
                    lo = max(g0, b_off)
                    hi = min(g0 + gw, b_off + b_cols)
                    if lo < hi:
                        if lo > g0:
                            nc.scalar.copy(s_sb[:, g0:lo], ps[:, :lo - g0])
                        nc.vector.tensor_add(
                            s_sb[:, lo:hi], ps[:, lo - g0:hi - g0],
                            bias_sb[:, lo - b_off:hi - b_off])
                        if hi < g0 + gw:
                            nc.scalar.copy(s_sb[:, hi:g0 + gw],
                                           ps[:, hi - g0:gw])
                    else:
                        nc.scalar.copy(s_sb[:, g0:g0 + gw], ps[:, :gw])
                    nc.vector.reduce_max(pmax[:, g:g + 1], s_sb[:, g0:g0 + gw],
                                         axis=AX)
                negm = statp.tile([P, 1], F32, tag="negm", name="negm")
                nc.vector.reduce_max(negm[:], pmax[:, :ngroups], axis=AX,
                                     negate=True)
                negm32 = statp.tile([P, 1], F32, tag="negm32", name="negm32")
                nc.vector.tensor_scalar_mul(negm32[:], negm[:], 32.0)
                state[i] = (s_sb, negm32)

            def stage_b(i):
                """exp + row-sum, P^T transposes, AV accumulation, 1/sum
                scale, output DMA."""
                s_cols, _, _ = cfg[i]
                stiles = s_cols // P
                ngroups = (s_cols + 511) // 512
                s_sb, negm32 = state.pop(i)

                p_sb = pp.tile([P, s_cols], AV_DT, tag="p", name="p_sb")
                gsum = statp.tile([P, ngroups], F32, tag="gsum", name="gsum")
                for g in range(ngroups):
                    g0 = g * 512
                    gw = min(512, s_cols - g0)
                    nc.scalar.activation(
                        p_sb[:, g0:g0 + gw], s_sb[:, g0:g0 + gw], EXP,
                        bias=negm32[:], scale=32.0,
                        accum_out=gsum[:, g:g + 1])
                rsum = statp.tile([P, 1], F32, tag="rsum", name="rsum")
                nc.vector.reduce_sum(rsum[:], gsum[:, :ngroups], axis=AX)
                rinv = statp.tile([P, 1], F32, tag="rinv", name="rinv")
                nc.vector.reciprocal(rinv[:], rsum[:])

                ps_o = ps_op.tile([P, D], F32, tag="o", name="ps_o")
                for st in range(stiles):
                    ps_t = ps_tp.tile([P, P], AV_DT, tag="t", name="ps_t")
                    nc.tensor.transpose(ps_t[:], p_sb[:, st * P:(st + 1) * P],
                                        ident[:])
                    pt_sb = ptp.tile([P, P], AV_DT, tag="pt", name="pt_sb")
                    nc.vector.tensor_copy(pt_sb[:], ps_t[:])
                    for dh in range(2):
                        nc.tensor.matmul(
                            ps_o[:, dh * 512:(dh + 1) * 512],
                            pt_sb[:],
                            v_sb[st][:, dh * 512:(dh + 1) * 512],
                            start=(st == 0), stop=(st == stiles - 1))
                o_sb = outp.tile([P, D], F32, tag="o_sb", name="o_sb")
                # scale + store per d-half so the first half's DMA overlaps
                # the second half's scale (shortens the kernel tail)
                for dh in range(2):
                    dsl = slice(dh * 512, (dh + 1) * 512)
                    nc.vector.tensor_scalar_mul(o_sb[:, dsl], ps_o[:, dsl],
                                                rinv[:])
                    nc.sync.dma_start(out[i * P:(i + 1) * P, dsl],
                                      o_sb[:, dsl])

            # Software pipeline: QK of one tile runs (on PE) while the
            # previous tile does softmax/exp on ACT/DVE, so PE never waits
            # on the softmax.  Tile 2 is moved last so the un-overlapped
            # final B stage is a small one (6 key blocks instead of 16).
            order = [0, 1, 3, 4, 5, 7, 6, 2]
            dma_a(order[0])
            for idx in range(len(order) + 1):
                if idx < len(order):
                    # issue the NEXT tile's DMAs first so its operands are
                    # in flight while this tile's QK runs
                    if idx + 1 < len(order):
                        dma_a(order[idx + 1])
                    compute_a(order[idx])
                if idx > 0:
                    stage_b(order[idx - 1])

    nc.compile()
    return nc


def _rows(causal: bool, p: int) -> np.ndarray:
    if causal:
        return np.concatenate(
            [256 * i + 2 * np.arange(P) + p for i in range(NQT)])
    return p * (NQT * P) + np.arange(NQT * P)


def _get(causal: bool):
    if causal not in _cache:
        _cache[causal] = _build(causal)
    return _cache[causal]


def kernel(query, key, value, mask):
    query = np.asarray(query, dtype=np.float32)
    key = np.asarray(key, dtype=np.float32)
    value = np.asarray(value, dtype=np.float32)
    mask = np.asarray(mask, dtype=np.float32)

    causal = bool(
        np.array_equal(mask, np.triu(np.ones((T, T), np.float32), k=1)))
    nc = _get(causal)
    cfg = _tile_cfg(causal)
    # bias folded pre-scale: 32*(S + mask*NEG/32) == 32*S + mask*NEG exactly
    mask_scaled = mask * np.float32(NEG / 32.0)

    def hilo_packed(x):
        # [D, n] fp32 -> [D, 2, n] fp16 with planes (hi, lo)
        hi = x.astype(np.float16)
        lo = (x - hi.astype(np.float32)).astype(np.float16)
        return np.ascontiguousarray(np.stack([hi, lo], axis=1))

    if QK_SPLIT:
        kT_hl = [hilo_packed(np.ascontiguousarray(key[b].T))
                 for b in range(B)]
    else:
        kTs = [np.ascontiguousarray(key[b].T) for b in range(B)]
    in_maps = []
    rows_by_core = []
    for c in range(NCORES):
        b, p = c // 2, c % 2
        rows = _rows(causal, p)
        rows_by_core.append((b, rows))
        qT_c = np.ascontiguousarray(query[b][rows].T)
        if causal:
            _, boff, bcols = cfg[0]
            bias_c = mask_scaled[rows[0:P], boff:boff + bcols]
        else:
            bias_c = np.stack([
                mask_scaled[rows[i * P:(i + 1) * P], boff:boff + bcols]
                for i, (_, boff, bcols) in enumerate(cfg)])
        im = {
            "v": np.ascontiguousarray(value[b]).astype(
                np.float16 if AV_DT == F16 else np.float32),
            "bias": np.ascontiguousarray(bias_c),
        }
        if QK_SPLIT:
            im["qThl"] = hilo_packed(qT_c)
            im["kThl"] = kT_hl[b]
        else:
            im["qT"] = qT_c
            im["kT"] = kTs[b]
        in_maps.append(im)

    res = run_bass_kernel_spmd(nc, in_maps, core_ids=list(range(NCORES)))

    outp = np.empty((B, T, D), dtype=np.float32)
    for c in range(NCORES):
        b, rows = rows_by_core[c]
        outp[b][rows] = res.results[c]["out"]
    return outp
